# revision 1
# baseline (speedup 1.0000x reference)
"""Trainium2 Bass kernel for a 2-layer GATv2 encoder (nn_CG_GNN_Encoder).

kernel(**inputs) takes full inputs (x [20000,512] f32, edge_index [2,320000]
int64, weights) and returns the full [20000, 512] f32 output, across 8 cores.

v2 design (per core, dst-node sharded):
  - Host: balance dst nodes into 8 cores x 20 blocks x 125 nodes; per-block
    edge lists padded to e_blk; UNWEIGHTED one-hot scatter matrices built on
    host (pad edges get all-zero columns -> no masks needed); |att|*(2/3)
    folded into Wl/Wr columns; 4 augmented weight columns compute the
    separable logit term A[n,h] = sum_c sigma_c xl'[n,c] on the fly.
  - Phase A per layer: x tiles -> PE transpose -> matmuls -> xl/xr [.,516]
    (512 features + 4 aug cols), bias added during PSUM evacuation.
  - AllGather xl across 8 cores; xr stays local.
  - Edge phase per block: 2 batched indirect gathers (xl[src], xr[dst]),
    t = xl+xr (one DVE add), logits = t_aug + (sum_pos|t| - sum_neg|t|)
    via 8 abs-reduces, p = exp(0.6*lg) written straight into the value
    tile's appended p-columns, xa = xl*p via broadcast mult, then one
    one-hot matmul per (s, head-pair) accumulating values+denominators
    in PSUM; normalize, ELU between layers.
"""

import numpy as np
from ml_dtypes import bfloat16

import concourse.bacc as bacc
import concourse.bass as bass
import concourse.mybir as mybir
import concourse.tile as tile
from concourse.bass_utils import run_bass_kernel_spmd

F32 = mybir.dt.float32
BF16 = mybir.dt.bfloat16
I32 = mybir.dt.int32
I16 = mybir.dt.int16
AX = mybir.AxisListType
OP = mybir.AluOpType
ACT = mybir.ActivationFunctionType

N = 20000
H = 4
C = 128
IN = 512
HC = H * C
W = HC + 4            # feature cols + 4 aug (per-head separable term)
WP = 640              # DRAM row padding (dma_gather needs 256B-mult stride)
NEG = 0.2
NCORES = 8
NSH = N // NCORES     # 2500
DBLK = 125
NBLK = NSH // DBLK    # 20
ATT_EPS = 1e-10
K23 = 2.0 / 3.0


# ----------------------------------------------------------------------------
# Host-side preprocessing
# ----------------------------------------------------------------------------

def _preprocess_graph(edge_index):
    src = np.concatenate([edge_index[0], np.arange(N, dtype=np.int64)])
    dst = np.concatenate([edge_index[1], np.arange(N, dtype=np.int64)])
    deg = np.bincount(dst, minlength=N)

    nbins = NCORES * NBLK
    order = np.argsort(-deg, kind="stable")
    import heapq
    bin_load = np.zeros(nbins, np.int64)
    bin_fill = np.zeros(nbins, np.int64)
    assign = np.zeros(N, np.int64)
    heap = [(0, b) for b in range(nbins)]
    heapq.heapify(heap)
    for nid in order:
        while True:
            load, b = heapq.heappop(heap)
            if bin_fill[b] < DBLK:
                break
        assign[nid] = b
        bin_fill[b] += 1
        bin_load[b] = load + deg[nid]
        if bin_fill[b] < DBLK:
            heapq.heappush(heap, (bin_load[b], b))

    perm = np.argsort(assign * N + np.arange(N), kind="stable")
    inv_perm = np.empty(N, np.int64)
    inv_perm[perm] = np.arange(N)

    e_bin = assign[dst]
    e_dst_pos = inv_perm[dst]
    e_src_pos = inv_perm[src]
    max_per_bin = int(np.bincount(e_bin, minlength=nbins).max())
    e_blk = -(-max_per_bin // 128) * 128
    S = e_blk // 128

    order_e = np.argsort(e_bin, kind="stable")
    eb = e_bin[order_e]
    starts = np.searchsorted(eb, np.arange(nbins))
    ends = np.searchsorted(eb, np.arange(nbins), side="right")

    E16 = e_blk // 16
    src16 = np.zeros((NCORES, NBLK, 16, E16), np.int16)
    dst16 = np.zeros((NCORES, NBLK, 16, E16), np.int16)
    onehot = np.zeros((NCORES, 128, NBLK, S, DBLK), bfloat16)

    for b in range(nbins):
        core, blk = divmod(b, NBLK)
        sel = order_e[starts[b]:ends[b]]
        n = len(sel)
        pos = np.arange(n)
        src16[core, blk, pos % 16, pos // 16] = e_src_pos[sel]
        dst16[core, blk, pos % 16, pos // 16] = e_dst_pos[sel] % NSH
        onehot[core, pos % 128, blk, pos // 128, e_dst_pos[sel] % DBLK] = 1.0

    per_core = []
    for core in range(NCORES):
        # [NBLK, 16, E16] -> [16, NBLK*E16] -> replicate to 128 partitions
        s16 = src16[core].transpose(1, 0, 2).reshape(16, -1)
        d16 = dst16[core].transpose(1, 0, 2).reshape(16, -1)
        per_core.append(dict(
            src_idx=np.tile(s16, (8, 1)).copy(),
            dst_idx=np.tile(d16, (8, 1)).copy(),
            onehot=onehot[core].reshape(128, -1).copy(),
        ))
    return per_core, dict(e_blk=e_blk, perm=perm)


def _prep_weights(inputs):
    """Per layer, build augmented scaled weights.

    Column order: per head, positive-att cols then negative-att cols.
    Feature cols scaled by (2/3)*|att| (so pos-minus-neg abs-reduce gives
    (2/3)*T2); aug col 512+h = sum_c sigma_c * (|a_c| W[:,c]) gives T1.
    logit = T1 + (2/3)T2 then exp(scale=0.6).
    """
    out = {}
    npos = []
    col_perms = []
    for l in range(2):
        att = np.asarray(inputs[f"att{l}"], np.float32)
        cols = []
        np_l = []
        for h in range(H):
            pos = np.where(att[h] >= 0)[0]
            neg = np.where(att[h] < 0)[0]
            cols.append(h * C + np.concatenate([pos, neg]))
            np_l.append(len(pos))
        cols = np.concatenate(cols)
        absa = np.maximum(np.abs(att.reshape(HC)[cols]), ATT_EPS)
        sigma = np.sign(att.reshape(HC)[cols])
        sigma[sigma == 0] = 1.0
        col_perms.append(cols)
        npos.append(np_l)

        Wl = np.asarray(inputs[f"Wl{l}"], np.float32)
        Wr = np.asarray(inputs[f"Wr{l}"], np.float32)
        bl = np.asarray(inputs[f"bl{l}"], np.float32)
        br = np.asarray(inputs[f"br{l}"], np.float32)
        if l == 1:
            Wl = Wl[col_perms[0], :]
            Wr = Wr[col_perms[0], :]
        Wl = Wl[:, cols]
        Wr = Wr[:, cols]
        bl = bl[cols]
        br = br[cols]

        # scaled feature block + aug cols
        def aug(Wmat, bvec):
            Ws = Wmat * (K23 * absa)[None, :]
            bs = bvec * (K23 * absa)
            Wa = np.zeros((Wmat.shape[0], 4), np.float32)
            ba = np.zeros(4, np.float32)
            for h in range(H):
                sl = slice(h * C, (h + 1) * C)
                Wa[:, h] = (Wmat[:, sl] * (absa * sigma)[None, sl]).sum(1)
                ba[h] = (bvec[sl] * (absa * sigma)[sl]).sum()
            return (np.concatenate([Ws, Wa], 1).astype(bfloat16),
                    np.concatenate([bs, ba]).astype(np.float32))

        wl_a, bl_a = aug(Wl, bl)
        wr_a, br_a = aug(Wr, br)
        out[f"wl{l}"] = wl_a                     # [IN, 516] bf16
        out[f"wr{l}"] = wr_a
        inv = 1.0 / (K23 * absa)
        auxrows = np.zeros((4, W), np.float32)
        auxrows[0, :] = np.concatenate([bl_a[:HC] * 0 + bl_a[:HC], bl_a[HC:]])
        auxrows[0] = bl_a
        auxrows[1] = br_a
        auxrows[2, :HC] = inv
        auxrows[3, :HC] = np.asarray(inputs[f"bias{l}"], np.float32)[cols]
        out[f"aux{l}"] = auxrows
    return out, npos, col_perms


# ----------------------------------------------------------------------------
# Device kernel
# ----------------------------------------------------------------------------

def _build(e_blk, npos):
    S = e_blk // 128
    nc = bacc.Bacc("TRN2", target_bir_lowering=False, debug=False,
                   num_devices=NCORES)

    x_in = nc.dram_tensor("x_shard", [NSH, IN], F32, kind="ExternalInput")
    wl_d = [nc.dram_tensor(f"wl{l}", [IN, W], BF16, kind="ExternalInput")
            for l in range(2)]
    wr_d = [nc.dram_tensor(f"wr{l}", [IN, W], BF16, kind="ExternalInput")
            for l in range(2)]
    aux_d = [nc.dram_tensor(f"aux{l}", [4, W], F32, kind="ExternalInput")
             for l in range(2)]
    E16 = e_blk // 16
    srcidx_d = nc.dram_tensor("src_idx", [128, NBLK * E16], I16,
                              kind="ExternalInput")
    dstidx_d = nc.dram_tensor("dst_idx", [128, NBLK * E16], I16,
                              kind="ExternalInput")
    oh_d = nc.dram_tensor("onehot", [128, NBLK * S * DBLK], BF16,
                          kind="ExternalInput")
    out_d = nc.dram_tensor("out", [NSH, HC], F32, kind="ExternalOutput")

    from concourse.masks import make_identity

    with tile.TileContext(nc) as tc:
        with tc.tile_pool(name="dram", bufs=1, space="DRAM") as dram, \
             tc.tile_pool(name="const", bufs=1) as cp, \
             tc.tile_pool(name="pha", bufs=2) as wp, \
             tc.tile_pool(name="gath", bufs=2) as gp, \
             tc.tile_pool(name="sm", bufs=2) as sp, \
             tc.tile_pool(name="psum", bufs=2, space="PSUM") as pp:

            xl_sh = [dram.tile([NSH, WP], BF16, name=f"xl_sh{l}") for l in range(2)]
            xr_sh = [dram.tile([NSH, WP], BF16, name=f"xr_sh{l}") for l in range(2)]
            xl_full = [dram.tile([N, WP], BF16, name=f"xl_full{l}")
                       for l in range(2)]
            h_mid = dram.tile([NSH, HC], F32, name="h_mid")

            ident = cp.tile([DBLK, DBLK], BF16, name="ident")
            make_identity(nc, ident[:])

            si_t = cp.tile([128, NBLK * E16], I16, name="si_t")
            di_t = cp.tile([128, NBLK * E16], I16, name="di_t")
            nc.sync.dma_start(out=si_t[:], in_=srcidx_d[:])
            nc.sync.dma_start(out=di_t[:], in_=dstidx_d[:])

            for l in range(2):
                # ---- phase A ----------------------------------------------
                wl_t = cp.tile([128, 4, W], BF16, name="wl_t", tag="wl_t")
                wr_t = cp.tile([128, 4, W], BF16, name="wr_t", tag="wr_t")
                for k in range(4):
                    nc.sync.dma_start(out=wl_t[:, k, :],
                                      in_=wl_d[l][k * 128:(k + 1) * 128, :])
                    nc.sync.dma_start(out=wr_t[:, k, :],
                                      in_=wr_d[l][k * 128:(k + 1) * 128, :])
                aux_b = []
                for r in range(4):
                    row = cp.tile([1, W], F32, name=f"ar{r}", tag=f"ar{r}")
                    nc.sync.dma_start(out=row[:], in_=aux_d[l][r:r + 1, :])
                    bc = cp.tile([128, W], F32, name=f"ab{r}", tag=f"ab{r}")
                    nc.gpsimd.partition_broadcast(bc[:], row[:])
                    aux_b.append(bc)
                bl_b, br_b, invatt_b, bias_b = aux_b

                src_x = x_in if l == 0 else h_mid
                for t in range(NBLK):
                    x_t = wp.tile([DBLK, IN], BF16, name="x_t", tag="x_t")
                    nc.gpsimd.dma_start(
                        out=x_t[:], in_=src_x[t * DBLK:(t + 1) * DBLK, :])
                    xT = wp.tile([128, 4, DBLK], BF16, name="xT", tag="xT")
                    for k in range(4):
                        ps_tr = pp.tile([128, DBLK], BF16, name="ps_tr",
                                        tag="ps_a")
                        nc.tensor.transpose(out=ps_tr[:],
                                            in_=x_t[:, k * 128:(k + 1) * 128],
                                            identity=ident[:])
                        nc.scalar.copy(out=xT[:, k, :], in_=ps_tr[:])
                    ps_xl = pp.tile([DBLK, HC], F32, name="ps_xl", tag="ps_b")
                    ps_xr = pp.tile([DBLK, HC], F32, name="ps_xr", tag="ps_c")
                    ps_al = pp.tile([DBLK, 8], F32, name="ps_al",
                                    tag="ps_d", bufs=1)
                    for k in range(4):
                        nc.tensor.matmul(out=ps_xl[:], lhsT=xT[:, k, :],
                                         rhs=wl_t[:, k, 0:HC],
                                         start=(k == 0), stop=(k == 3))
                    for k in range(4):
                        nc.tensor.matmul(out=ps_xr[:], lhsT=xT[:, k, :],
                                         rhs=wr_t[:, k, 0:HC],
                                         start=(k == 0), stop=(k == 3))
                    for k in range(4):
                        nc.tensor.matmul(out=ps_al[:, 0:4], lhsT=xT[:, k, :],
                                         rhs=wl_t[:, k, HC:W],
                                         start=(k == 0), stop=(k == 3))
                    for k in range(4):
                        nc.tensor.matmul(out=ps_al[:, 4:8], lhsT=xT[:, k, :],
                                         rhs=wr_t[:, k, HC:W],
                                         start=(k == 0), stop=(k == 3))
                    xl_o = wp.tile([DBLK, W], BF16, name="xl_o", tag="xl_o")
                    xr_o = wp.tile([DBLK, W], BF16, name="xr_o", tag="xr_o")
                    nc.vector.tensor_add(out=xl_o[:, 0:HC], in0=ps_xl[:],
                                         in1=bl_b[:DBLK, 0:HC])
                    nc.vector.tensor_add(out=xr_o[:, 0:HC], in0=ps_xr[:],
                                         in1=br_b[:DBLK, 0:HC])
                    nc.vector.tensor_add(out=xl_o[:, HC:W], in0=ps_al[:, 0:4],
                                         in1=bl_b[:DBLK, HC:W])
                    nc.vector.tensor_add(out=xr_o[:, HC:W], in0=ps_al[:, 4:8],
                                         in1=br_b[:DBLK, HC:W])
                    nc.sync.dma_start(
                        out=xl_sh[l][t * DBLK:(t + 1) * DBLK, 0:W],
                        in_=xl_o[:])
                    nc.sync.dma_start(
                        out=xr_sh[l][t * DBLK:(t + 1) * DBLK, 0:W],
                        in_=xr_o[:])

                nc.gpsimd.collective_compute(
                    "AllGather", OP.bypass,
                    replica_groups=[list(range(NCORES))],
                    ins=[xl_sh[l][:]], outs=[xl_full[l][:]],
                )

                # ---- edge phase -------------------------------------------
                for b in range(NBLK):
                    oh_b = gp.tile([128, S, DBLK], BF16, name="oh_b",
                                   tag="oh_b")
                    nc.sync.dma_start(
                        out=oh_b[:],
                        in_=oh_d[:, b * S * DBLK:(b + 1) * S * DBLK])
                    xl_g = gp.tile([128, S, WP], BF16, name="xl_g",
                                   tag="xl_g")
                    xr_g = gp.tile([128, S, WP], BF16, name="xr_g",
                                   tag="xr_g")
                    for c0 in range(0, e_blk, 1024):
                        n = min(1024, e_blk - c0)
                        io0 = b * E16 + c0 // 16
                        nc.gpsimd.dma_gather(
                            out_ap=xl_g[:, c0 // 128:(c0 + n) // 128, :],
                            in_ap=xl_full[l][:],
                            idxs_ap=si_t[:, io0:io0 + n // 16],
                            num_idxs=n, num_idxs_reg=n, elem_size=WP,
                            single_packet=False, queue_num=0)
                        nc.gpsimd.dma_gather(
                            out_ap=xr_g[:, c0 // 128:(c0 + n) // 128, :],
                            in_ap=xr_sh[l][:],
                            idxs_ap=di_t[:, io0:io0 + n // 16],
                            num_idxs=n, num_idxs_reg=n, elem_size=WP,
                            single_packet=False, queue_num=0)

                    nc.vector.tensor_add(out=xr_g[:, :, 0:W],
                                         in0=xl_g[:, :, 0:W],
                                         in1=xr_g[:, :, 0:W])

                    # abs-reduces: lg_pn [128, 2(pn), S, 4(h)]
                    lg_pn = sp.tile([128, 2, S, H], F32, name="lg_pn",
                                    tag="lg_pn")
                    for h in range(H):
                        np_h = npos[l][h]
                        lo, mid, hi = h * C, h * C + np_h, (h + 1) * C
                        if np_h > 0:
                            nc.vector.tensor_reduce(
                                out=lg_pn[:, 0, :, h], in_=xr_g[:, :, lo:mid],
                                axis=AX.X, op=OP.add,
                                apply_absolute_value=True)
                        else:
                            nc.vector.memset(lg_pn[:, 0, :, h], 0.0)
                        if np_h < C:
                            nc.vector.tensor_reduce(
                                out=lg_pn[:, 1, :, h], in_=xr_g[:, :, mid:hi],
                                axis=AX.X, op=OP.add,
                                apply_absolute_value=True)
                        else:
                            nc.vector.memset(lg_pn[:, 1, :, h], 0.0)

                    lg = sp.tile([128, S, H], F32, name="lg", tag="lg")
                    nc.vector.tensor_tensor(out=lg[:], in0=lg_pn[:, 0, :, :],
                                            in1=lg_pn[:, 1, :, :],
                                            op=OP.subtract)
                    nc.vector.tensor_tensor(out=lg[:], in0=lg[:],
                                            in1=xr_g[:, :, HC:W],
                                            op=OP.add)

                    # xa: [128, S, 2, 258]; cols 256:258 of each hp get p
                    xa = gp.tile([128, S, 2, 258], BF16, name="xa", tag="xa")
                    nc.scalar.activation(
                        out=xa[:, :, :, 256:258],
                        in_=lg[:].rearrange("p s (a b) -> p s a b", a=2),
                        func=ACT.Exp, scale=0.6)
                    for hp in range(2):
                        nc.vector.tensor_tensor(
                            out=xa[:, :, hp, 0:256].rearrange(
                                "p s (h c) -> p s h c", h=2),
                            in0=xl_g[:, :, hp * 256:(hp + 1) * 256].rearrange(
                                "p s (h c) -> p s h c", h=2),
                            in1=xa[:, :, hp, 256:258][:, :, :, None]
                                .to_broadcast([128, S, 2, 128]),
                            op=OP.mult)

                    ps_of = [pp.tile([DBLK, HC], F32, name=f"ps_o{hp}",
                                     tag=f"ps_{'bc'[hp]}") for hp in range(2)]
                    ps_o = [t[:, 0:258] for t in ps_of]
                    for s in range(S):
                        oh_s = oh_b[:, s, :]
                        for hp in range(2):
                            nc.tensor.matmul(out=ps_o[hp][:], lhsT=oh_s,
                                             rhs=xa[:, s, hp, :],
                                             start=(s == 0), stop=(s == S - 1))

                    rinv = sp.tile([DBLK, 4], F32, name="rinv", tag="rinv")
                    for hp in range(2):
                        nc.vector.reciprocal(out=rinv[:, 2 * hp:2 * hp + 2],
                                             in_=ps_o[hp][:, 256:258])
                    o_sb = sp.tile([DBLK, HC], F32, name="o_sb", tag="o_sb")
                    for hp in range(2):
                        nc.vector.tensor_tensor(
                            out=o_sb[:, hp * 256:(hp + 1) * 256].rearrange(
                                "p (h c) -> p h c", h=2),
                            in0=ps_o[hp][:, 0:256].rearrange(
                                "p (h c) -> p h c", h=2),
                            in1=rinv[:, 2 * hp:2 * hp + 2][:, :, None]
                                .to_broadcast([DBLK, 2, 128]),
                            op=OP.mult)
                    nc.vector.tensor_mul(out=o_sb[:], in0=o_sb[:],
                                         in1=invatt_b[:DBLK, 0:HC])
                    nc.vector.tensor_add(out=o_sb[:], in0=o_sb[:],
                                         in1=bias_b[:DBLK, 0:HC])
                    rows = slice(b * DBLK, (b + 1) * DBLK)
                    if l == 0:
                        r_t = sp.tile([DBLK, HC], F32, name="r_t", tag="r_t")
                        nc.scalar.activation(out=r_t[:], in_=o_sb[:],
                                             func=ACT.Relu)
                        e_t = sp.tile([DBLK, HC], F32, name="e_t", tag="e_t")
                        nc.scalar.activation(out=e_t[:], in_=o_sb[:],
                                             func=ACT.Exp)
                        nc.vector.tensor_scalar(
                            out=e_t[:], in0=e_t[:], scalar1=-1.0, scalar2=0.0,
                            op0=OP.add, op1=OP.min)
                        nc.vector.tensor_add(out=r_t[:], in0=r_t[:],
                                             in1=e_t[:])
                        nc.sync.dma_start(out=h_mid[rows, :], in_=r_t[:])
                    else:
                        nc.sync.dma_start(out=out_d[rows, :], in_=o_sb[:])

    nc.compile()
    return nc


_CACHE = {}


def _get_nc(e_blk, npos_key):
    key = (e_blk, npos_key)
    if key not in _CACHE:
        _CACHE[key] = _build(e_blk, [list(npos_key[0]), list(npos_key[1])])
    return _CACHE[key]


def kernel(**inputs):
    per_core, meta = _preprocess_graph(np.asarray(inputs["edge_index"]))
    wprep, npos, col_perms = _prep_weights(inputs)
    e_blk = meta["e_blk"]
    perm = meta["perm"]

    nc = _get_nc(e_blk, (tuple(npos[0]), tuple(npos[1])))

    x = np.asarray(inputs["x"], np.float32)
    x_perm = x[perm]
    in_maps = []
    for core in range(NCORES):
        m = dict(
            x_shard=np.ascontiguousarray(x_perm[core * NSH:(core + 1) * NSH]),
            src_idx=per_core[core]["src_idx"],
            dst_idx=per_core[core]["dst_idx"],
            onehot=per_core[core]["onehot"],
        )
        for l in range(2):
            m[f"wl{l}"] = wprep[f"wl{l}"]
            m[f"wr{l}"] = wprep[f"wr{l}"]
            m[f"aux{l}"] = wprep[f"aux{l}"]
        in_maps.append(m)

    trace = bool(inputs.pop("_trace", False))
    res = run_bass_kernel_spmd(nc, in_maps, core_ids=list(range(NCORES)),
                               trace=trace)
    out_rows = np.concatenate([res.results[c]["out"] for c in range(NCORES)],
                              axis=0)
    out = np.zeros((N, HC), np.float32)
    tmp = np.zeros((N, HC), np.float32)
    tmp[perm] = out_rows
    out[:, col_perms[1]] = tmp
    if trace:
        kernel._last_result = res
    return out



# revision 6
# speedup vs baseline: 1.1888x; 1.1888x over previous
"""Trainium2 Bass kernel for a 2-layer GATv2 encoder (nn_CG_GNN_Encoder).

kernel(**inputs) takes full inputs (x [20000,512] f32, edge_index [2,320000]
int64, weights) and returns the full [20000, 512] f32 output, across 8 cores.

v3 design (per core, dst-node sharded):
  - Host: balance dst nodes into 8 cores x 20 blocks x 125 nodes; per-block
    edge lists padded to e_blk; one-hot scatter matrices in BOTH orientations
    (edge-major `oh` for value aggregation, dst-major `ohT` for broadcasting
    dst features to edges); |att| magnitudes folded into Wl/Wr columns with
    pos-att columns ordered before neg-att per head.
  - Phase A per layer: x chunks DMA-transposed, 8 matmuls per 125-node tile
    -> xl/xr [., 512], bias added during PSUM evacuation; xl stored to DRAM
    and AllGathered in 4 chunks (overlapping phase A); xr stays local.
  - Edge phase per block: ONE batched indirect gather (xl[src]) split across
    2 SWDGE queues; xr[dst] broadcast on the tensor engine (ohT matmul) with
    xl accumulated via identity matmul; LeakyReLU applied by the scalar
    engine during PSUM->SBUF evacuation.  Per-head logits = pos-column sum
    minus neg-column sum (DVE reduces), p = exp(logit); p duplicated into
    adjacent column pairs so the value multiply runs in the DVE packed 2x
    mode; one-hot matmuls accumulate values + denominators in PSUM;
    normalize, ELU between layers.  Layer-1 output unscale/bias on host.
"""

import numpy as np
from ml_dtypes import bfloat16

import concourse.bacc as bacc
import concourse.bass as bass
import concourse.mybir as mybir
import concourse.tile as tile
from concourse.bass_utils import run_bass_kernel_spmd

F32 = mybir.dt.float32
BF16 = mybir.dt.bfloat16
I16 = mybir.dt.int16
AX = mybir.AxisListType
OP = mybir.AluOpType
ACT = mybir.ActivationFunctionType

N = 20000
H = 4
C = 128
IN = 512
HC = H * C            # 512
NEG = 0.2
NCORES = 8
NSH = N // NCORES     # 2500
DBLK = 125
NBLK = NSH // DBLK    # 20
NPAD = NBLK * 128     # 2560 padded rows (tile t at rows 128t..128t+124)
ATT_EPS = 1e-10


# ----------------------------------------------------------------------------
# Host-side preprocessing
# ----------------------------------------------------------------------------

def _preprocess_graph(edge_index):
    src = np.concatenate([edge_index[0], np.arange(N, dtype=np.int64)])
    dst = np.concatenate([edge_index[1], np.arange(N, dtype=np.int64)])
    deg = np.bincount(dst, minlength=N)

    nbins = NCORES * NBLK
    order = np.argsort(-deg, kind="stable")
    import heapq
    bin_load = np.zeros(nbins, np.int64)
    bin_fill = np.zeros(nbins, np.int64)
    assign = np.zeros(N, np.int64)
    heap = [(0, b) for b in range(nbins)]
    heapq.heapify(heap)
    for nid in order:
        while True:
            load, b = heapq.heappop(heap)
            if bin_fill[b] < DBLK:
                break
        assign[nid] = b
        bin_fill[b] += 1
        bin_load[b] = load + deg[nid]
        if bin_fill[b] < DBLK:
            heapq.heappush(heap, (bin_load[b], b))

    perm = np.argsort(assign * N + np.arange(N), kind="stable")
    inv_perm = np.empty(N, np.int64)
    inv_perm[perm] = np.arange(N)

    e_bin = assign[dst]
    e_dst_pos = inv_perm[dst]
    e_src_pos = inv_perm[src]
    max_per_bin = int(np.bincount(e_bin, minlength=nbins).max())
    e_blk = -(-max_per_bin // 128) * 128
    S = e_blk // 128

    order_e = np.argsort(e_bin, kind="stable")
    eb = e_bin[order_e]
    starts = np.searchsorted(eb, np.arange(nbins))
    ends = np.searchsorted(eb, np.arange(nbins), side="right")

    E16 = e_blk // 16
    src16 = np.zeros((NCORES, NBLK, 16, E16), np.int16)
    onehot = np.zeros((NCORES, 128, NBLK, S, DBLK), bfloat16)
    onehotT = np.zeros((NCORES, DBLK, NBLK, S, 128), bfloat16)

    for b in range(nbins):
        core, blk = divmod(b, NBLK)
        sel = order_e[starts[b]:ends[b]]
        n = len(sel)
        pos = np.arange(n)
        d_loc = e_dst_pos[sel] % DBLK
        # xl_full is chunk-major: AllGather chunk c (625 rows per core) is
        # contiguous as [8 cores, 625].  Map src position -> xl_full row.
        sp_ = e_src_pos[sel]
        s_core, s_r = sp_ // NSH, sp_ % NSH
        src_row = (s_r // 625) * (NCORES * 625) + s_core * 625 + (s_r % 625)
        src16[core, blk, pos % 16, pos // 16] = src_row
        onehot[core, pos % 128, blk, pos // 128, d_loc] = 1.0
        onehotT[core, d_loc, blk, pos // 128, pos % 128] = 1.0

    per_core = []
    for core in range(NCORES):
        s16 = src16[core].transpose(1, 0, 2).reshape(16, -1)
        per_core.append(dict(
            src_idx=np.tile(s16, (8, 1)).copy(),
            onehot=onehot[core].reshape(128, -1).copy(),
            onehotT=onehotT[core].reshape(DBLK, -1).copy(),
        ))
    return per_core, dict(e_blk=e_blk, perm=perm)


def _prep_weights(inputs):
    """Per layer: permute columns pos-att-first per head, scale columns by
    max(|att|, eps).  logit = sum_pos lrelu(col) - sum_neg lrelu(col)."""
    out = {}
    npos = []
    col_perms = []
    invs = []
    for l in range(2):
        att = np.asarray(inputs[f"att{l}"], np.float32)
        cols = []
        np_l = []
        for h in range(H):
            pos = np.where(att[h] >= 0)[0]
            neg = np.where(att[h] < 0)[0]
            cols.append(h * C + np.concatenate([pos, neg]))
            np_l.append(len(pos))
        cols = np.concatenate(cols)
        absa = np.maximum(np.abs(att.reshape(HC)[cols]), ATT_EPS)
        col_perms.append(cols)
        npos.append(np_l)
        invs.append((1.0 / absa).astype(np.float32))

        Wl = np.asarray(inputs[f"Wl{l}"], np.float32)
        Wr = np.asarray(inputs[f"Wr{l}"], np.float32)
        bl = np.asarray(inputs[f"bl{l}"], np.float32)
        br = np.asarray(inputs[f"br{l}"], np.float32)
        if l == 1:
            Wl = Wl[col_perms[0], :]
            Wr = Wr[col_perms[0], :]
        out[f"wl{l}"] = (Wl[:, cols] * absa[None, :]).astype(bfloat16)
        out[f"wr{l}"] = (Wr[:, cols] * absa[None, :]).astype(bfloat16)
        aux = np.zeros((4, HC), np.float32)
        aux[0] = bl[cols] * absa
        aux[1] = br[cols] * absa
        aux[2] = invs[l]
        aux[3] = np.asarray(inputs[f"bias{l}"], np.float32)[cols]
        out[f"aux{l}"] = aux.astype(bfloat16)
    return out, npos, col_perms, invs


# ----------------------------------------------------------------------------
# Device kernel
# ----------------------------------------------------------------------------

def _build(e_blk, npos):
    S = e_blk // 128
    E16 = e_blk // 16
    S0 = (S + 1) // 2          # chunks handled by queue 0
    n0 = S0 * 128
    n1 = e_blk - n0
    nc = bacc.Bacc("TRN2", target_bir_lowering=False, debug=False,
                   num_devices=NCORES, num_swdge_queues=2)

    x_in = nc.dram_tensor("x_pad", [NPAD, IN], BF16, kind="ExternalInput")
    wl_d = [nc.dram_tensor(f"wl{l}", [IN, HC], BF16, kind="ExternalInput")
            for l in range(2)]
    wr_d = [nc.dram_tensor(f"wr{l}", [IN, HC], BF16, kind="ExternalInput")
            for l in range(2)]
    aux_d = [nc.dram_tensor(f"aux{l}", [4, HC], BF16, kind="ExternalInput")
             for l in range(2)]
    srcidx_d = nc.dram_tensor("src_idx", [128, NBLK * E16], I16,
                              kind="ExternalInput")
    oh_d = nc.dram_tensor("onehot", [128, NBLK * S * DBLK], BF16,
                          kind="ExternalInput")
    ohT_d = nc.dram_tensor("onehotT", [DBLK, NBLK * S * 128], BF16,
                           kind="ExternalInput")
    out_d = nc.dram_tensor("out", [NSH, HC], F32, kind="ExternalOutput")

    from concourse.masks import make_identity

    with tile.TileContext(nc) as tc:
        with tc.tile_pool(name="dram", bufs=1, space="DRAM") as dram, \
             tc.tile_pool(name="const", bufs=1) as cp, \
             tc.tile_pool(name="pha", bufs=3) as wp, \
             tc.tile_pool(name="gath", bufs=2) as gp, \
             tc.tile_pool(name="sm", bufs=2) as sp, \
             tc.tile_pool(name="psum", bufs=2, space="PSUM") as pp:

            xl_sh = [dram.tile([NSH, HC], BF16, name=f"xl_sh{l}")
                     for l in range(2)]
            xr_dr = [dram.tile([NSH, HC], BF16, name=f"xr_dr{l}")
                     for l in range(2)]
            xl_full = [dram.tile([N, HC], BF16, name=f"xl_full{l}")
                       for l in range(2)]
            h_pad = dram.tile([NPAD, HC], BF16, name="h_pad")

            ident = cp.tile([128, 128], BF16, name="ident")
            make_identity(nc, ident[:])

            si_t = cp.tile([128, NBLK * E16], I16, name="si_t")
            nc.sync.dma_start(out=si_t[:], in_=srcidx_d[:])

            # weights + aux broadcast tiles (both layers, resident)
            wl_t, wr_t, aux_b = [], [], []
            for l in range(2):
                wlt = cp.tile([128, 4, HC], BF16, name=f"wl_t{l}")
                wrt = cp.tile([128, 4, HC], BF16, name=f"wr_t{l}")
                for k in range(4):
                    nc.sync.dma_start(out=wlt[:, k, :],
                                      in_=wl_d[l][k * 128:(k + 1) * 128, :])
                    nc.sync.dma_start(out=wrt[:, k, :],
                                      in_=wr_d[l][k * 128:(k + 1) * 128, :])
                wl_t.append(wlt)
                wr_t.append(wrt)
                rows = []
                for r in range(4):
                    if l == 1 and r >= 2:
                        rows.append(None)
                        continue
                    row = cp.tile([1, HC], BF16, name=f"ar{l}{r}")
                    nc.sync.dma_start(out=row[:], in_=aux_d[l][r:r + 1, :])
                    bc = cp.tile([128, HC], BF16, name=f"ab{l}{r}")
                    nc.gpsimd.partition_broadcast(bc[:], row[:])
                    rows.append(bc)
                aux_b.append(rows)

            # ---- emit helpers ---------------------------------------------
            def phase_a_tile(l, t):
                src_pad = x_in if l == 0 else h_pad
                xT = wp.tile([128, 4, 128], BF16, name="xT", tag="xT")
                for k in range(4):
                    nc.sync.dma_start_transpose(
                        out=xT[:, k, :],
                        in_=src_pad[t * 128:(t + 1) * 128,
                                    k * 128:(k + 1) * 128])
                ps_xl = pp.tile([DBLK, HC], F32, name="ps_xl", tag="ps_xl",
                                bufs=1)
                ps_xr = pp.tile([DBLK, HC], F32, name="ps_xr", tag="ps_xr",
                                bufs=1)
                for k in range(4):
                    nc.tensor.matmul(out=ps_xl[:], lhsT=xT[:, k, 0:DBLK],
                                     rhs=wl_t[l][:, k, :],
                                     start=(k == 0), stop=(k == 3))
                for k in range(4):
                    nc.tensor.matmul(out=ps_xr[:], lhsT=xT[:, k, 0:DBLK],
                                     rhs=wr_t[l][:, k, :],
                                     start=(k == 0), stop=(k == 3))
                xl_o = wp.tile([DBLK, HC], BF16, name="xl_o", tag="xl_o")
                xr_o = wp.tile([DBLK, HC], BF16, name="xr_o", tag="xr_o")
                nc.vector.tensor_add(out=xl_o[:], in0=ps_xl[:],
                                     in1=aux_b[l][0][:DBLK, :])
                nc.vector.tensor_add(out=xr_o[:], in0=ps_xr[:],
                                     in1=aux_b[l][1][:DBLK, :])
                rows = slice(t * DBLK, (t + 1) * DBLK)
                nc.sync.dma_start(out=xl_sh[l][rows, :], in_=xl_o[:])
                nc.sync.dma_start(out=xr_dr[l][rows, :], in_=xr_o[:])

            def ag_chunk(l, c):
                nc.gpsimd.collective_compute(
                    "AllGather", OP.bypass,
                    replica_groups=[list(range(NCORES))],
                    ins=[xl_sh[l][c * 625:(c + 1) * 625, :]],
                    outs=[xl_full[l][c * NCORES * 625:(c + 1) * NCORES * 625,
                                     :]],
                )

            def edge_block(l, b):
                xl_g = gp.tile([128, S, HC], BF16, name="xl_g", tag="xl_g")
                io0 = b * E16
                nc.gpsimd.dma_gather(
                    out_ap=xl_g[:, 0:S0, :], in_ap=xl_full[l][:],
                    idxs_ap=si_t[:, io0:io0 + n0 // 16],
                    num_idxs=n0, num_idxs_reg=n0, elem_size=HC,
                    single_packet=False, queue_num=0)
                nc.gpsimd.dma_gather(
                    out_ap=xl_g[:, S0:S, :], in_ap=xl_full[l][:],
                    idxs_ap=si_t[:, io0 + n0 // 16:io0 + E16],
                    num_idxs=n1, num_idxs_reg=n1, elem_size=HC,
                    single_packet=False, queue_num=1)

                oh_b = gp.tile([128, S, DBLK], BF16, name="oh_b", tag="oh_b")
                nc.sync.dma_start(
                    out=oh_b[:],
                    in_=oh_d[:, b * S * DBLK:(b + 1) * S * DBLK])
                ohT_b = gp.tile([DBLK, S, 128], BF16, name="ohT_b",
                                tag="ohT_b")
                nc.sync.dma_start(
                    out=ohT_b[:],
                    in_=ohT_d[:, b * S * 128:(b + 1) * S * 128])
                xr_b = gp.tile([DBLK, HC], BF16, name="xr_b", tag="xr_b")
                nc.sync.dma_start(
                    out=xr_b[:], in_=xr_dr[l][b * DBLK:(b + 1) * DBLK, :])

                # t = xr[dst] + xl[src] on PE; LeakyReLU during ACT evac
                t_lr = gp.tile([128, S, HC], BF16, name="t_lr", tag="t_lr")
                for s2 in range(0, S, 2):
                    w = min(2, S - s2)
                    ps_t = pp.tile([128, 2, HC], F32, name="ps_t", tag="ps_t")
                    for s in range(s2, s2 + w):
                        nc.tensor.matmul(out=ps_t[:, s - s2, :],
                                         lhsT=ohT_b[:, s, :], rhs=xr_b[:],
                                         start=True, stop=False)
                        nc.tensor.matmul(out=ps_t[:, s - s2, :],
                                         lhsT=ident[:], rhs=xl_g[:, s, :],
                                         start=False, stop=True)
                    nc.scalar.activation(out=t_lr[:, s2:s2 + w, :],
                                         in_=ps_t[:, 0:w, :],
                                         func=ACT.Prelu, alpha=NEG)

                # logits: pos-sum minus neg-sum per head
                lg_pn = sp.tile([128, 2, S, H], F32, name="lg_pn",
                                tag="lg_pn")
                for h in range(H):
                    np_h = npos[l][h]
                    lo, mid, hi = h * C, h * C + np_h, (h + 1) * C
                    if np_h > 0:
                        nc.vector.tensor_reduce(
                            out=lg_pn[:, 0, :, h], in_=t_lr[:, :, lo:mid],
                            axis=AX.X, op=OP.add)
                    else:
                        nc.vector.memset(lg_pn[:, 0, :, h], 0.0)
                    if np_h < C:
                        nc.vector.tensor_reduce(
                            out=lg_pn[:, 1, :, h], in_=t_lr[:, :, mid:hi],
                            axis=AX.X, op=OP.add)
                    else:
                        nc.vector.memset(lg_pn[:, 1, :, h], 0.0)
                lg = sp.tile([128, S, H], F32, name="lg", tag="lg")
                nc.vector.tensor_tensor(out=lg[:], in0=lg_pn[:, 0, :, :],
                                        in1=lg_pn[:, 1, :, :],
                                        op=OP.subtract)

                # p = exp(lg): into xa denominator cols + duplicated pairs
                xa = gp.tile([128, S, 2, 258], BF16, name="xa", tag="xa")
                nc.scalar.activation(
                    out=xa[:, :, :, 256:258],
                    in_=lg[:].rearrange("p s (a b) -> p s a b", a=2),
                    func=ACT.Exp)
                p_dup = sp.tile([128, S, H, 2], BF16, name="p_dup",
                                tag="p_dup")
                nc.scalar.activation(out=p_dup[:, :, :, 0], in_=lg[:],
                                     func=ACT.Exp)
                nc.scalar.activation(out=p_dup[:, :, :, 1], in_=lg[:],
                                     func=ACT.Exp)

                # xa = xl * p  (packed 2x: p pairs along last dim)
                for h in range(H):
                    hp, hh = divmod(h, 2)
                    nc.vector.tensor_tensor(
                        out=xa[:, :, hp, hh * 128:(hh + 1) * 128].rearrange(
                            "p s (pr two) -> p s pr two", two=2),
                        in0=xl_g[:, :, h * 128:(h + 1) * 128].rearrange(
                            "p s (pr two) -> p s pr two", two=2),
                        in1=p_dup[:, :, h, None, :]
                            .to_broadcast([128, S, 64, 2]),
                        op=OP.mult)

                ps_o = [pp.tile([DBLK, 258], F32, name=f"ps_o{hp}",
                                tag=f"ps_o{hp}", bufs=1) for hp in range(2)]
                for s in range(S):
                    for hp in range(2):
                        nc.tensor.matmul(out=ps_o[hp][:], lhsT=oh_b[:, s, :],
                                         rhs=xa[:, s, hp, :],
                                         start=(s == 0), stop=(s == S - 1))

                rinv = sp.tile([DBLK, 4], F32, name="rinv", tag="rinv")
                for hp in range(2):
                    nc.vector.reciprocal(out=rinv[:, 2 * hp:2 * hp + 2],
                                         in_=ps_o[hp][:, 256:258])
                o_sb = sp.tile([DBLK, HC], BF16 if l == 0 else F32,
                               name="o_sb", tag=f"o_sb{l}")
                for hp in range(2):
                    nc.vector.tensor_tensor(
                        out=o_sb[:, hp * 256:(hp + 1) * 256].rearrange(
                            "p (h c) -> p h c", h=2),
                        in0=ps_o[hp][:, 0:256].rearrange(
                            "p (h c) -> p h c", h=2),
                        in1=rinv[:, 2 * hp:2 * hp + 2][:, :, None]
                            .to_broadcast([DBLK, 2, 128]),
                        op=OP.mult)
                if l == 0:
                    nc.vector.tensor_mul(out=o_sb[:], in0=o_sb[:],
                                         in1=aux_b[0][2][:DBLK, :])
                    nc.vector.tensor_add(out=o_sb[:], in0=o_sb[:],
                                         in1=aux_b[0][3][:DBLK, :])
                    r_t = sp.tile([DBLK, HC], BF16, name="r_t", tag="r_t")
                    nc.scalar.activation(out=r_t[:], in_=o_sb[:],
                                         func=ACT.Relu)
                    e_t = sp.tile([DBLK, HC], BF16, name="e_t", tag="e_t")
                    nc.scalar.activation(out=e_t[:], in_=o_sb[:],
                                         func=ACT.Exp)
                    nc.vector.tensor_scalar(
                        out=e_t[:], in0=e_t[:], scalar1=-1.0, scalar2=0.0,
                        op0=OP.add, op1=OP.min)
                    h_t = sp.tile([DBLK, HC], BF16, name="h_t", tag="h_t")
                    nc.vector.tensor_add(out=h_t[:], in0=r_t[:], in1=e_t[:])
                    nc.sync.dma_start(
                        out=h_pad[b * 128:b * 128 + DBLK, :], in_=h_t[:])
                else:
                    nc.sync.dma_start(
                        out=out_d[b * DBLK:(b + 1) * DBLK, :], in_=o_sb[:])

            # ---- schedule -------------------------------------------------
            for t in range(NBLK):
                phase_a_tile(0, t)
                if t % 5 == 4:
                    ag_chunk(0, t // 5)
            for b in range(NBLK):
                edge_block(0, b)
                phase_a_tile(1, b)
                if b % 5 == 4:
                    ag_chunk(1, b // 5)
            for b in range(NBLK):
                edge_block(1, b)

    nc.compile()
    return nc


_CACHE = {}


def _get_nc(e_blk, npos_key):
    key = (e_blk, npos_key)
    if key not in _CACHE:
        _CACHE[key] = _build(e_blk, [list(npos_key[0]), list(npos_key[1])])
    return _CACHE[key]


def kernel(**inputs):
    per_core, meta = _preprocess_graph(np.asarray(inputs["edge_index"]))
    wprep, npos, col_perms, invs = _prep_weights(inputs)
    e_blk = meta["e_blk"]
    perm = meta["perm"]

    nc = _get_nc(e_blk, (tuple(npos[0]), tuple(npos[1])))

    x = np.asarray(inputs["x"], np.float32)
    x_perm = x[perm].astype(bfloat16)
    in_maps = []
    for core in range(NCORES):
        xp = np.zeros((NPAD, IN), bfloat16)
        xc = x_perm[core * NSH:(core + 1) * NSH]
        xp.reshape(NBLK, 128, IN)[:, :DBLK, :] = xc.reshape(NBLK, DBLK, IN)
        m = dict(
            x_pad=xp,
            src_idx=per_core[core]["src_idx"],
            onehot=per_core[core]["onehot"],
            onehotT=per_core[core]["onehotT"],
        )
        for l in range(2):
            m[f"wl{l}"] = wprep[f"wl{l}"]
            m[f"wr{l}"] = wprep[f"wr{l}"]
            m[f"aux{l}"] = wprep[f"aux{l}"]
        in_maps.append(m)

    trace = bool(inputs.pop("_trace", False))
    res = run_bass_kernel_spmd(nc, in_maps, core_ids=list(range(NCORES)),
                               trace=trace)
    out_rows = np.concatenate([res.results[c]["out"] for c in range(NCORES)],
                              axis=0)
    tmp = np.zeros((N, HC), np.float32)
    tmp[perm] = out_rows
    out = np.zeros((N, HC), np.float32)
    bias1 = np.asarray(inputs["bias1"], np.float32)
    out[:, col_perms[1]] = tmp * invs[1][None, :] + bias1[col_perms[1]][None, :]
    if trace:
        kernel._last_result = res
    return out


# revision 8
# speedup vs baseline: 1.3606x; 1.1446x over previous
"""Trainium2 Bass kernel for a 2-layer GATv2 encoder (nn_CG_GNN_Encoder).

kernel(**inputs) takes full inputs (x [20000,512] f32, edge_index [2,320000]
int64, weights) and returns the full [20000, 512] f32 output, across 8 cores.

v3 design (per core, dst-node sharded):
  - Host: balance dst nodes into 8 cores x 20 blocks x 125 nodes; per-block
    edge lists padded to e_blk; one-hot scatter matrices in BOTH orientations
    (edge-major `oh` for value aggregation, dst-major `ohT` for broadcasting
    dst features to edges); |att| magnitudes folded into Wl/Wr columns with
    pos-att columns ordered before neg-att per head.
  - Phase A per layer: x chunks DMA-transposed, 8 matmuls per 125-node tile
    -> xl/xr [., 512], bias added during PSUM evacuation; xl stored to DRAM
    and AllGathered in 4 chunks (overlapping phase A); xr stays local.
  - Edge phase per block: ONE batched indirect gather (xl[src]) split across
    2 SWDGE queues; xr[dst] broadcast on the tensor engine (ohT matmul) with
    xl accumulated via identity matmul; LeakyReLU applied by the scalar
    engine during PSUM->SBUF evacuation.  Per-head logits = pos-column sum
    minus neg-column sum (DVE reduces), p = exp(logit); p duplicated into
    adjacent column pairs so the value multiply runs in the DVE packed 2x
    mode; one-hot matmuls accumulate values + denominators in PSUM;
    normalize, ELU between layers.  Layer-1 output unscale/bias on host.
"""

import numpy as np
from ml_dtypes import bfloat16

import concourse.bacc as bacc
import concourse.bass as bass
import concourse.mybir as mybir
import concourse.tile as tile
from concourse.bass_utils import run_bass_kernel_spmd

F32 = mybir.dt.float32
BF16 = mybir.dt.bfloat16
I16 = mybir.dt.int16
AX = mybir.AxisListType
OP = mybir.AluOpType
ACT = mybir.ActivationFunctionType

N = 20000
H = 4
C = 128
IN = 512
HC = H * C            # 512
NEG = 0.2
NCORES = 8
NSH = N // NCORES     # 2500
DBLK = 125
NBLK = NSH // DBLK    # 20
NPAD = NBLK * 128     # 2560 padded rows (tile t at rows 128t..128t+124)
ATT_EPS = 1e-10


# ----------------------------------------------------------------------------
# Host-side preprocessing
# ----------------------------------------------------------------------------

def _preprocess_graph(edge_index):
    src = np.concatenate([edge_index[0], np.arange(N, dtype=np.int64)])
    dst = np.concatenate([edge_index[1], np.arange(N, dtype=np.int64)])
    deg = np.bincount(dst, minlength=N)

    nbins = NCORES * NBLK
    order = np.argsort(-deg, kind="stable")
    import heapq
    bin_load = np.zeros(nbins, np.int64)
    bin_fill = np.zeros(nbins, np.int64)
    assign = np.zeros(N, np.int64)
    heap = [(0, b) for b in range(nbins)]
    heapq.heapify(heap)
    for nid in order:
        while True:
            load, b = heapq.heappop(heap)
            if bin_fill[b] < DBLK:
                break
        assign[nid] = b
        bin_fill[b] += 1
        bin_load[b] = load + deg[nid]
        if bin_fill[b] < DBLK:
            heapq.heappush(heap, (bin_load[b], b))

    perm = np.argsort(assign * N + np.arange(N), kind="stable")
    inv_perm = np.empty(N, np.int64)
    inv_perm[perm] = np.arange(N)

    e_bin = assign[dst]
    e_dst_pos = inv_perm[dst]
    e_src_pos = inv_perm[src]
    max_per_bin = int(np.bincount(e_bin, minlength=nbins).max())
    e_blk = -(-max_per_bin // 128) * 128
    S = e_blk // 128

    order_e = np.argsort(e_bin, kind="stable")
    eb = e_bin[order_e]
    starts = np.searchsorted(eb, np.arange(nbins))
    ends = np.searchsorted(eb, np.arange(nbins), side="right")

    E16 = e_blk // 16
    src16 = np.zeros((NCORES, NBLK, 16, E16), np.int16)
    onehot = np.zeros((NCORES, 128, NBLK, S, DBLK), bfloat16)
    onehotT = np.zeros((NCORES, DBLK, NBLK, S, 128), bfloat16)

    for b in range(nbins):
        core, blk = divmod(b, NBLK)
        sel = order_e[starts[b]:ends[b]]
        n = len(sel)
        pos = np.arange(n)
        d_loc = e_dst_pos[sel] % DBLK
        # xl_full is chunk-major: AllGather chunk c (625 rows per core) is
        # contiguous as [8 cores, 625].  Map src position -> xl_full row.
        sp_ = e_src_pos[sel]
        s_core, s_r = sp_ // NSH, sp_ % NSH
        src_row = (s_r // 625) * (NCORES * 625) + s_core * 625 + (s_r % 625)
        src16[core, blk, pos % 16, pos // 16] = src_row
        onehot[core, pos % 128, blk, pos // 128, d_loc] = 1.0
        onehotT[core, d_loc, blk, pos // 128, pos % 128] = 1.0

    per_core = []
    for core in range(NCORES):
        s16 = src16[core].transpose(1, 0, 2).reshape(16, -1)
        per_core.append(dict(
            src_idx=np.tile(s16, (8, 1)).copy(),
            onehot=onehot[core].reshape(128, -1).copy(),
            onehotT=onehotT[core].reshape(DBLK, -1).copy(),
        ))
    return per_core, dict(e_blk=e_blk, perm=perm)


def _prep_weights(inputs):
    """Per layer: permute columns pos-att-first per head, scale columns by
    max(|att|, eps).  logit = sum_pos lrelu(col) - sum_neg lrelu(col)."""
    out = {}
    npos = []
    col_perms = []
    invs = []
    for l in range(2):
        att = np.asarray(inputs[f"att{l}"], np.float32)
        cols = []
        np_l = []
        for h in range(H):
            pos = np.where(att[h] >= 0)[0]
            neg = np.where(att[h] < 0)[0]
            cols.append(h * C + np.concatenate([pos, neg]))
            np_l.append(len(pos))
        cols = np.concatenate(cols)
        absa = np.maximum(np.abs(att.reshape(HC)[cols]), ATT_EPS)
        col_perms.append(cols)
        npos.append(np_l)
        invs.append((1.0 / absa).astype(np.float32))

        Wl = np.asarray(inputs[f"Wl{l}"], np.float32)
        Wr = np.asarray(inputs[f"Wr{l}"], np.float32)
        bl = np.asarray(inputs[f"bl{l}"], np.float32)
        br = np.asarray(inputs[f"br{l}"], np.float32)
        if l == 1:
            Wl = Wl[col_perms[0], :]
            Wr = Wr[col_perms[0], :]
        out[f"wl{l}"] = (Wl[:, cols] * absa[None, :]).astype(bfloat16)
        out[f"wr{l}"] = (Wr[:, cols] * absa[None, :]).astype(bfloat16)
        aux = np.zeros((4, HC), np.float32)
        aux[0] = bl[cols] * absa
        aux[1] = br[cols] * absa
        aux[2] = invs[l]
        aux[3] = np.asarray(inputs[f"bias{l}"], np.float32)[cols]
        out[f"aux{l}"] = aux.astype(bfloat16)
    return out, npos, col_perms, invs


# ----------------------------------------------------------------------------
# Device kernel
# ----------------------------------------------------------------------------

def _build(e_blk, npos):
    S = e_blk // 128
    E16 = e_blk // 16
    S0 = (S + 1) // 2          # chunks handled by queue 0
    n0 = S0 * 128
    n1 = e_blk - n0
    nc = bacc.Bacc("TRN2", target_bir_lowering=False, debug=False,
                   num_devices=NCORES, num_swdge_queues=2)

    x_in = nc.dram_tensor("x_pad", [NPAD, IN], BF16, kind="ExternalInput")
    wl_d = [nc.dram_tensor(f"wl{l}", [IN, HC], BF16, kind="ExternalInput")
            for l in range(2)]
    wr_d = [nc.dram_tensor(f"wr{l}", [IN, HC], BF16, kind="ExternalInput")
            for l in range(2)]
    aux_d = [nc.dram_tensor(f"aux{l}", [4, HC], BF16, kind="ExternalInput")
             for l in range(2)]
    srcidx_d = nc.dram_tensor("src_idx", [128, NBLK * E16], I16,
                              kind="ExternalInput")
    oh_d = nc.dram_tensor("onehot", [128, NBLK * S * DBLK], BF16,
                          kind="ExternalInput")
    ohT_d = nc.dram_tensor("onehotT", [DBLK, NBLK * S * 128], BF16,
                           kind="ExternalInput")
    out_d = nc.dram_tensor("out", [NSH, HC], F32, kind="ExternalOutput")

    from concourse.masks import make_identity

    with tile.TileContext(nc) as tc:
        with tc.tile_pool(name="dram", bufs=1, space="DRAM") as dram, \
             tc.tile_pool(name="const", bufs=1) as cp, \
             tc.tile_pool(name="pha", bufs=3) as wp, \
             tc.tile_pool(name="gath", bufs=2) as gp, \
             tc.tile_pool(name="sm", bufs=2) as sp, \
             tc.tile_pool(name="psum", bufs=2, space="PSUM") as pp:

            xl_sh = [dram.tile([NSH, HC], BF16, name=f"xl_sh{l}")
                     for l in range(2)]
            xr_dr = [dram.tile([NSH, HC], BF16, name=f"xr_dr{l}")
                     for l in range(2)]
            xl_full = [dram.tile([N, HC], BF16, name=f"xl_full{l}")
                       for l in range(2)]
            h_pad = dram.tile([NPAD, HC], BF16, name="h_pad")

            ident = cp.tile([128, 128], BF16, name="ident")
            make_identity(nc, ident[:])

            si_t = cp.tile([128, NBLK * E16], I16, name="si_t")
            nc.sync.dma_start(out=si_t[:], in_=srcidx_d[:])

            # weights + aux broadcast tiles (both layers, resident)
            wl_t, wr_t, aux_b = [], [], []
            for l in range(2):
                wlt = cp.tile([128, 4, HC], BF16, name=f"wl_t{l}")
                wrt = cp.tile([128, 4, HC], BF16, name=f"wr_t{l}")
                for k in range(4):
                    nc.sync.dma_start(out=wlt[:, k, :],
                                      in_=wl_d[l][k * 128:(k + 1) * 128, :])
                    nc.sync.dma_start(out=wrt[:, k, :],
                                      in_=wr_d[l][k * 128:(k + 1) * 128, :])
                wl_t.append(wlt)
                wr_t.append(wrt)
                rows = []
                for r in range(4):
                    if l == 1 and r >= 2:
                        rows.append(None)
                        continue
                    row = cp.tile([1, HC], BF16, name=f"ar{l}{r}")
                    nc.sync.dma_start(out=row[:], in_=aux_d[l][r:r + 1, :])
                    bc = cp.tile([128, HC], BF16, name=f"ab{l}{r}")
                    nc.gpsimd.partition_broadcast(bc[:], row[:])
                    rows.append(bc)
                aux_b.append(rows)

            # ---- emit helpers ---------------------------------------------
            def phase_a_group(l, g):
                """Tiles 5g..5g+4 (640 padded rows) of phase A for layer l."""
                src_pad = x_in if l == 0 else h_pad
                xT = wp.tile([128, 4, 640], BF16, name="xT", tag="xT")
                for k in range(4):
                    nc.sync.dma_start_transpose(
                        out=xT[:, k, :],
                        in_=src_pad[g * 640:(g + 1) * 640,
                                    k * 128:(k + 1) * 128])
                for t in range(5 * g, 5 * g + 5):
                    off = (t - 5 * g) * 128
                    ps_xl = pp.tile([DBLK, HC], F32, name="ps_xl",
                                    tag="ps_sh", bufs=2)
                    for k in range(4):
                        nc.tensor.matmul(
                            out=ps_xl[:], lhsT=xT[:, k, off:off + DBLK],
                            rhs=wl_t[l][:, k, :],
                            start=(k == 0), stop=(k == 3))
                    xl_o = wp.tile([DBLK, HC], BF16, name="xl_o", tag="xl_o")
                    nc.vector.tensor_add(out=xl_o[:], in0=ps_xl[:],
                                         in1=aux_b[l][0][:DBLK, :])
                    rows = slice(t * DBLK, (t + 1) * DBLK)
                    nc.sync.dma_start(out=xl_sh[l][rows, :], in_=xl_o[:])
                    ps_xr = pp.tile([DBLK, HC], F32, name="ps_xr",
                                    tag="ps_sh", bufs=2)
                    for k in range(4):
                        nc.tensor.matmul(
                            out=ps_xr[:], lhsT=xT[:, k, off:off + DBLK],
                            rhs=wr_t[l][:, k, :],
                            start=(k == 0), stop=(k == 3))
                    xr_o = wp.tile([DBLK, HC], BF16, name="xr_o", tag="xr_o")
                    nc.vector.tensor_add(out=xr_o[:], in0=ps_xr[:],
                                         in1=aux_b[l][1][:DBLK, :])
                    nc.sync.dma_start(out=xr_dr[l][rows, :], in_=xr_o[:])

            def ag_chunk(l, c):
                nc.gpsimd.collective_compute(
                    "AllGather", OP.bypass,
                    replica_groups=[list(range(NCORES))],
                    ins=[xl_sh[l][c * 625:(c + 1) * 625, :]],
                    outs=[xl_full[l][c * NCORES * 625:(c + 1) * NCORES * 625,
                                     :]],
                )

            def edge_A(l, b):
                """Gathers + streams + t-matmuls for block b.  Returns the
                tiles needed by the evac/B stages."""
                xl_g = gp.tile([128, S, HC], BF16, name="xl_g", tag="xl_g")
                io0 = b * E16
                nc.gpsimd.dma_gather(
                    out_ap=xl_g[:, 0:S0, :], in_ap=xl_full[l][:],
                    idxs_ap=si_t[:, io0:io0 + n0 // 16],
                    num_idxs=n0, num_idxs_reg=n0, elem_size=HC,
                    single_packet=False, queue_num=0)
                nc.gpsimd.dma_gather(
                    out_ap=xl_g[:, S0:S, :], in_ap=xl_full[l][:],
                    idxs_ap=si_t[:, io0 + n0 // 16:io0 + E16],
                    num_idxs=n1, num_idxs_reg=n1, elem_size=HC,
                    single_packet=False, queue_num=1)

                oh_b = gp.tile([128, S, DBLK], BF16, name="oh_b", tag="oh_b")
                nc.sync.dma_start(
                    out=oh_b[:],
                    in_=oh_d[:, b * S * DBLK:(b + 1) * S * DBLK])
                ohT_b = gp.tile([DBLK, S, 128], BF16, name="ohT_b",
                                tag="ohT_b")
                nc.sync.dma_start(
                    out=ohT_b[:],
                    in_=ohT_d[:, b * S * 128:(b + 1) * S * 128])
                xr_b = gp.tile([DBLK, HC], BF16, name="xr_b", tag="xr_b")
                nc.sync.dma_start(
                    out=xr_b[:], in_=xr_dr[l][b * DBLK:(b + 1) * DBLK, :])

                ps_list = []
                for s2 in range(0, S, 2):
                    w = min(2, S - s2)
                    ps_t = pp.tile([128, 2, HC], F32, name="ps_t", tag="ps_t",
                                   bufs=3)
                    for s in range(s2, s2 + w):
                        nc.tensor.matmul(out=ps_t[:, s - s2, :],
                                         lhsT=ohT_b[:, s, :], rhs=xr_b[:],
                                         start=True, stop=False)
                        nc.tensor.matmul(out=ps_t[:, s - s2, :],
                                         lhsT=ident[:], rhs=xl_g[:, s, :],
                                         start=False, stop=True)
                    ps_list.append((s2, w, ps_t))
                return xl_g, oh_b, ps_list

            def edge_evac(l, b, st):
                """ACT LeakyReLU evacuation PSUM -> bf16 SBUF."""
                xl_g, oh_b, ps_list = st
                t_lr = gp.tile([128, S, HC], BF16, name="t_lr", tag="t_lr")
                for s2, w, ps_t in ps_list:
                    nc.scalar.activation(out=t_lr[:, s2:s2 + w, :],
                                         in_=ps_t[:, 0:w, :],
                                         func=ACT.Prelu, alpha=NEG)
                return t_lr

            def edge_B1(l, b, st, t_lr):
                """Logit reduces + exp (DVE + small ACT)."""
                lg_pn = sp.tile([128, 2, S, H], F32, name="lg_pn",
                                tag="lg_pn")
                for h in range(H):
                    np_h = npos[l][h]
                    lo, mid, hi = h * C, h * C + np_h, (h + 1) * C
                    if np_h > 0:
                        nc.vector.tensor_reduce(
                            out=lg_pn[:, 0, :, h], in_=t_lr[:, :, lo:mid],
                            axis=AX.X, op=OP.add)
                    else:
                        nc.vector.memset(lg_pn[:, 0, :, h], 0.0)
                    if np_h < C:
                        nc.vector.tensor_reduce(
                            out=lg_pn[:, 1, :, h], in_=t_lr[:, :, mid:hi],
                            axis=AX.X, op=OP.add)
                    else:
                        nc.vector.memset(lg_pn[:, 1, :, h], 0.0)
                lg = sp.tile([128, S, H], F32, name="lg", tag="lg")
                nc.vector.tensor_tensor(out=lg[:], in0=lg_pn[:, 0, :, :],
                                        in1=lg_pn[:, 1, :, :],
                                        op=OP.subtract)

                xa = gp.tile([128, S, 2, 258], BF16, name="xa", tag="xa")
                nc.scalar.activation(
                    out=xa[:, :, :, 256:258],
                    in_=lg[:].rearrange("p s (a b) -> p s a b", a=2),
                    func=ACT.Exp)
                p_dup = sp.tile([128, S, H, 2], BF16, name="p_dup",
                                tag="p_dup")
                nc.scalar.activation(out=p_dup[:, :, :, 0], in_=lg[:],
                                     func=ACT.Exp)
                nc.scalar.activation(out=p_dup[:, :, :, 1], in_=lg[:],
                                     func=ACT.Exp)
                return xa, p_dup

            def edge_B2(l, b, st, xa, p_dup):
                """Value multiply + one-hot matmuls + normalize + store."""
                xl_g, oh_b, ps_list = st
                for h in range(H):
                    hp, hh = divmod(h, 2)
                    nc.vector.tensor_tensor(
                        out=xa[:, :, hp, hh * 128:(hh + 1) * 128].rearrange(
                            "p s (pr two) -> p s pr two", two=2),
                        in0=xl_g[:, :, h * 128:(h + 1) * 128].rearrange(
                            "p s (pr two) -> p s pr two", two=2),
                        in1=p_dup[:, :, h, None, :]
                            .to_broadcast([128, S, 64, 2]),
                        op=OP.mult)

                ps_o = [pp.tile([DBLK, HC], F32, name=f"ps_o{hp}",
                                tag="ps_sh", bufs=2) for hp in range(2)]
                for s in range(S):
                    for hp in range(2):
                        nc.tensor.matmul(out=ps_o[hp][:, 0:258],
                                         lhsT=oh_b[:, s, :],
                                         rhs=xa[:, s, hp, :],
                                         start=(s == 0), stop=(s == S - 1))

                rinv = sp.tile([DBLK, 4], F32, name="rinv", tag="rinv")
                for hp in range(2):
                    nc.vector.reciprocal(out=rinv[:, 2 * hp:2 * hp + 2],
                                         in_=ps_o[hp][:, 256:258])
                o_sb = sp.tile([DBLK, HC], BF16 if l == 0 else F32,
                               name="o_sb", tag=f"o_sb{l}")
                for hp in range(2):
                    nc.vector.tensor_tensor(
                        out=o_sb[:, hp * 256:(hp + 1) * 256].rearrange(
                            "p (h c) -> p h c", h=2),
                        in0=ps_o[hp][:, 0:256].rearrange(
                            "p (h c) -> p h c", h=2),
                        in1=rinv[:, 2 * hp:2 * hp + 2][:, :, None]
                            .to_broadcast([DBLK, 2, 128]),
                        op=OP.mult)
                if l == 0:
                    nc.vector.tensor_mul(out=o_sb[:], in0=o_sb[:],
                                         in1=aux_b[0][2][:DBLK, :])
                    nc.vector.tensor_add(out=o_sb[:], in0=o_sb[:],
                                         in1=aux_b[0][3][:DBLK, :])
                    r_t = sp.tile([DBLK, HC], BF16, name="r_t", tag="r_t")
                    nc.scalar.activation(out=r_t[:], in_=o_sb[:],
                                         func=ACT.Relu)
                    e_t = sp.tile([DBLK, HC], BF16, name="e_t", tag="e_t")
                    nc.scalar.activation(out=e_t[:], in_=o_sb[:],
                                         func=ACT.Exp)
                    nc.vector.tensor_scalar(
                        out=e_t[:], in0=e_t[:], scalar1=-1.0, scalar2=0.0,
                        op0=OP.add, op1=OP.min)
                    h_t = sp.tile([DBLK, HC], BF16, name="h_t", tag="h_t")
                    nc.vector.tensor_add(out=h_t[:], in0=r_t[:], in1=e_t[:])
                    nc.sync.dma_start(
                        out=h_pad[b * 128:b * 128 + DBLK, :], in_=h_t[:])
                else:
                    nc.sync.dma_start(
                        out=out_d[b * DBLK:(b + 1) * DBLK, :], in_=o_sb[:])

            # ---- schedule (1-block software pipeline per layer) -----------
            def emit_layer_edges(l, between=None):
                st = edge_A(l, 0)
                t_lr = edge_evac(l, 0, st)
                prev = (0, st, t_lr)
                for b in range(1, NBLK + 1):
                    pb, pst, pt = prev
                    nxt = None
                    if b < NBLK:
                        nxt = edge_A(l, b)
                    xa, p_dup = edge_B1(l, pb, pst, pt)
                    if nxt is not None:
                        nt = edge_evac(l, b, nxt)
                    edge_B2(l, pb, pst, xa, p_dup)
                    if between is not None:
                        between(pb)
                    if nxt is not None:
                        prev = (b, nxt, nt)

            for g in range(4):
                phase_a_group(0, g)
                ag_chunk(0, g)

            def _between_l0(b):
                if b % 5 == 4:
                    phase_a_group(1, b // 5)
                    ag_chunk(1, b // 5)

            emit_layer_edges(0, between=_between_l0)
            emit_layer_edges(1)

    nc.compile()
    return nc


_CACHE = {}


def _get_nc(e_blk, npos_key):
    key = (e_blk, npos_key)
    if key not in _CACHE:
        _CACHE[key] = _build(e_blk, [list(npos_key[0]), list(npos_key[1])])
    return _CACHE[key]


def kernel(**inputs):
    per_core, meta = _preprocess_graph(np.asarray(inputs["edge_index"]))
    wprep, npos, col_perms, invs = _prep_weights(inputs)
    e_blk = meta["e_blk"]
    perm = meta["perm"]

    nc = _get_nc(e_blk, (tuple(npos[0]), tuple(npos[1])))

    x = np.asarray(inputs["x"], np.float32)
    x_perm = x[perm].astype(bfloat16)
    in_maps = []
    for core in range(NCORES):
        xp = np.zeros((NPAD, IN), bfloat16)
        xc = x_perm[core * NSH:(core + 1) * NSH]
        xp.reshape(NBLK, 128, IN)[:, :DBLK, :] = xc.reshape(NBLK, DBLK, IN)
        m = dict(
            x_pad=xp,
            src_idx=per_core[core]["src_idx"],
            onehot=per_core[core]["onehot"],
            onehotT=per_core[core]["onehotT"],
        )
        for l in range(2):
            m[f"wl{l}"] = wprep[f"wl{l}"]
            m[f"wr{l}"] = wprep[f"wr{l}"]
            m[f"aux{l}"] = wprep[f"aux{l}"]
        in_maps.append(m)

    trace = bool(inputs.pop("_trace", False))
    res = run_bass_kernel_spmd(nc, in_maps, core_ids=list(range(NCORES)),
                               trace=trace)
    out_rows = np.concatenate([res.results[c]["out"] for c in range(NCORES)],
                              axis=0)
    tmp = np.zeros((N, HC), np.float32)
    tmp[perm] = out_rows
    out = np.zeros((N, HC), np.float32)
    bias1 = np.asarray(inputs["bias1"], np.float32)
    out[:, col_perms[1]] = tmp * invs[1][None, :] + bias1[col_perms[1]][None, :]
    if trace:
        kernel._last_result = res
    return out


# revision 10
# speedup vs baseline: 1.5538x; 1.1420x over previous
"""Trainium2 Bass kernel for a 2-layer GATv2 encoder (nn_CG_GNN_Encoder).

kernel(**inputs) takes full inputs (x [20000,512] f32, edge_index [2,320000]
int64, weights) and returns the full [20000, 512] f32 output, across 8 cores.

v3 design (per core, dst-node sharded):
  - Host: balance dst nodes into 8 cores x 20 blocks x 125 nodes; per-block
    edge lists padded to e_blk; one-hot scatter matrices in BOTH orientations
    (edge-major `oh` for value aggregation, dst-major `ohT` for broadcasting
    dst features to edges); |att| magnitudes folded into Wl/Wr columns with
    pos-att columns ordered before neg-att per head.
  - Phase A per layer: x chunks DMA-transposed, 8 matmuls per 125-node tile
    -> xl/xr [., 512], bias added during PSUM evacuation; xl stored to DRAM
    and AllGathered in 4 chunks (overlapping phase A); xr stays local.
  - Edge phase per block: ONE batched indirect gather (xl[src]) split across
    2 SWDGE queues; xr[dst] broadcast on the tensor engine (ohT matmul) with
    xl accumulated via identity matmul; LeakyReLU applied by the scalar
    engine during PSUM->SBUF evacuation.  Per-head logits = pos-column sum
    minus neg-column sum (DVE reduces), p = exp(logit); p duplicated into
    adjacent column pairs so the value multiply runs in the DVE packed 2x
    mode; one-hot matmuls accumulate values + denominators in PSUM;
    normalize, ELU between layers.  Layer-1 output unscale/bias on host.
"""

import numpy as np
from ml_dtypes import bfloat16

import concourse.bacc as bacc
import concourse.bass as bass
import concourse.mybir as mybir
import concourse.tile as tile
from concourse.bass_utils import run_bass_kernel_spmd

F32 = mybir.dt.float32
BF16 = mybir.dt.bfloat16
I16 = mybir.dt.int16
AX = mybir.AxisListType
OP = mybir.AluOpType
ACT = mybir.ActivationFunctionType

N = 20000
H = 4
C = 128
IN = 512
HC = H * C            # 512
NEG = 0.2
NCORES = 8
NSH = N // NCORES     # 2500
DBLK = 125
NBLK = NSH // DBLK    # 20
NPAD = NBLK * 128     # 2560 padded rows (tile t at rows 128t..128t+124)
ATT_EPS = 1e-10


# ----------------------------------------------------------------------------
# Host-side preprocessing
# ----------------------------------------------------------------------------

def _preprocess_graph(edge_index):
    src = np.concatenate([edge_index[0], np.arange(N, dtype=np.int64)])
    dst = np.concatenate([edge_index[1], np.arange(N, dtype=np.int64)])
    deg = np.bincount(dst, minlength=N)

    nbins = NCORES * NBLK
    order = np.argsort(-deg, kind="stable")
    import heapq
    bin_load = np.zeros(nbins, np.int64)
    bin_fill = np.zeros(nbins, np.int64)
    assign = np.zeros(N, np.int64)
    heap = [(0, b) for b in range(nbins)]
    heapq.heapify(heap)
    for nid in order:
        while True:
            load, b = heapq.heappop(heap)
            if bin_fill[b] < DBLK:
                break
        assign[nid] = b
        bin_fill[b] += 1
        bin_load[b] = load + deg[nid]
        if bin_fill[b] < DBLK:
            heapq.heappush(heap, (bin_load[b], b))

    perm = np.argsort(assign * N + np.arange(N), kind="stable")
    inv_perm = np.empty(N, np.int64)
    inv_perm[perm] = np.arange(N)

    e_bin = assign[dst]
    e_dst_pos = inv_perm[dst]
    e_src_pos = inv_perm[src]
    max_per_bin = int(np.bincount(e_bin, minlength=nbins).max())
    e_blk = -(-max_per_bin // 128) * 128
    S = e_blk // 128

    order_e = np.argsort(e_bin, kind="stable")
    eb = e_bin[order_e]
    starts = np.searchsorted(eb, np.arange(nbins))
    ends = np.searchsorted(eb, np.arange(nbins), side="right")

    E16 = e_blk // 16
    src16 = np.zeros((NCORES, NBLK, 16, E16), np.int16)
    onehot = np.zeros((NCORES, 128, NBLK, S, DBLK), bfloat16)
    onehotT = np.zeros((NCORES, DBLK, NBLK, S, 128), bfloat16)

    for b in range(nbins):
        core, blk = divmod(b, NBLK)
        sel = order_e[starts[b]:ends[b]]
        n = len(sel)
        pos = np.arange(n)
        d_loc = e_dst_pos[sel] % DBLK
        # xl_full is chunk-major: AllGather chunk c (625 rows per core) is
        # contiguous as [8 cores, 625].  Map src position -> xl_full row.
        sp_ = e_src_pos[sel]
        s_core, s_r = sp_ // NSH, sp_ % NSH
        src_row = (s_r // 625) * (NCORES * 625) + s_core * 625 + (s_r % 625)
        src16[core, blk, pos % 16, pos // 16] = src_row
        onehot[core, pos % 128, blk, pos // 128, d_loc] = 1.0
        onehotT[core, d_loc, blk, pos // 128, pos % 128] = 1.0

    per_core = []
    for core in range(NCORES):
        s16 = src16[core].transpose(1, 0, 2).reshape(16, -1)
        per_core.append(dict(
            src_idx=np.tile(s16, (8, 1)).copy(),
            onehot=onehot[core].reshape(128, -1).copy(),
            onehotT=onehotT[core].reshape(DBLK, -1).copy(),
        ))
    return per_core, dict(e_blk=e_blk, perm=perm)


def _prep_weights(inputs):
    """Per layer: permute columns pos-att-first per head, scale columns by
    max(|att|, eps).  logit = sum_pos lrelu(col) - sum_neg lrelu(col)."""
    out = {}
    npos = []
    col_perms = []
    invs = []
    for l in range(2):
        att = np.asarray(inputs[f"att{l}"], np.float32)
        cols = []
        np_l = []
        for h in range(H):
            pos = np.where(att[h] >= 0)[0]
            neg = np.where(att[h] < 0)[0]
            cols.append(h * C + np.concatenate([pos, neg]))
            np_l.append(len(pos))
        cols = np.concatenate(cols)
        absa = np.maximum(np.abs(att.reshape(HC)[cols]), ATT_EPS)
        col_perms.append(cols)
        npos.append(np_l)
        invs.append((1.0 / absa).astype(np.float32))

        Wl = np.asarray(inputs[f"Wl{l}"], np.float32)
        Wr = np.asarray(inputs[f"Wr{l}"], np.float32)
        bl = np.asarray(inputs[f"bl{l}"], np.float32)
        br = np.asarray(inputs[f"br{l}"], np.float32)
        if l == 1:
            Wl = Wl[col_perms[0], :]
            Wr = Wr[col_perms[0], :]
        out[f"wl{l}"] = (Wl[:, cols] * absa[None, :]).astype(bfloat16)
        out[f"wr{l}"] = (Wr[:, cols] * absa[None, :]).astype(bfloat16)
        aux = np.zeros((4, HC), np.float32)
        aux[0] = bl[cols] * absa
        aux[1] = br[cols] * absa
        aux[2] = invs[l]
        aux[3] = np.asarray(inputs[f"bias{l}"], np.float32)[cols]
        out[f"aux{l}"] = aux.astype(bfloat16)
    return out, npos, col_perms, invs


# ----------------------------------------------------------------------------
# Device kernel
# ----------------------------------------------------------------------------

def _build(e_blk, npos):
    S = e_blk // 128
    E16 = e_blk // 16
    S0 = (S + 1) // 2          # chunks handled by queue 0
    n0 = S0 * 128
    n1 = e_blk - n0
    nc = bacc.Bacc("TRN2", target_bir_lowering=False, debug=False,
                   num_devices=NCORES, num_swdge_queues=2)

    x_in = nc.dram_tensor("x_pad", [NPAD, IN], BF16, kind="ExternalInput")
    wl_d = [nc.dram_tensor(f"wl{l}", [IN, HC], BF16, kind="ExternalInput")
            for l in range(2)]
    wr_d = [nc.dram_tensor(f"wr{l}", [IN, HC], BF16, kind="ExternalInput")
            for l in range(2)]
    aux_d = [nc.dram_tensor(f"aux{l}", [4, HC], BF16, kind="ExternalInput")
             for l in range(2)]
    srcidx_d = nc.dram_tensor("src_idx", [128, NBLK * E16], I16,
                              kind="ExternalInput")
    oh_d = nc.dram_tensor("onehot", [128, NBLK * S * DBLK], BF16,
                          kind="ExternalInput")
    ohT_d = nc.dram_tensor("onehotT", [DBLK, NBLK * S * 128], BF16,
                           kind="ExternalInput")
    out_d = nc.dram_tensor("out", [NSH, HC], F32, kind="ExternalOutput")

    from concourse.masks import make_identity

    with tile.TileContext(nc) as tc:
        with tc.tile_pool(name="dram", bufs=1, space="DRAM") as dram, \
             tc.tile_pool(name="const", bufs=1) as cp, \
             tc.tile_pool(name="pha", bufs=3) as wp, \
             tc.tile_pool(name="gath", bufs=2) as gp, \
             tc.tile_pool(name="sm", bufs=2) as sp, \
             tc.tile_pool(name="psum", bufs=2, space="PSUM") as pp:

            xl_sh = [dram.tile([NSH, HC], BF16, name=f"xl_sh{l}")
                     for l in range(2)]
            xr_dr = [dram.tile([NSH, HC], BF16, name=f"xr_dr{l}")
                     for l in range(2)]
            xl_full = [dram.tile([N, HC], BF16, name=f"xl_full{l}")
                       for l in range(2)]
            h_pad = dram.tile([NPAD, HC], BF16, name="h_pad")

            ident = cp.tile([128, 128], BF16, name="ident")
            make_identity(nc, ident[:])

            si_t = cp.tile([128, NBLK * E16], I16, name="si_t")
            nc.sync.dma_start(out=si_t[:], in_=srcidx_d[:])

            # weights + aux broadcast tiles (both layers, resident)
            wl_t, wr_t, aux_b = [], [], []
            for l in range(2):
                wlt = cp.tile([128, 4, HC], BF16, name=f"wl_t{l}")
                wrt = cp.tile([128, 4, HC], BF16, name=f"wr_t{l}")
                for k in range(4):
                    nc.sync.dma_start(out=wlt[:, k, :],
                                      in_=wl_d[l][k * 128:(k + 1) * 128, :])
                    nc.sync.dma_start(out=wrt[:, k, :],
                                      in_=wr_d[l][k * 128:(k + 1) * 128, :])
                wl_t.append(wlt)
                wr_t.append(wrt)
                rows = []
                for r in range(4):
                    if l == 1 and r >= 2:
                        rows.append(None)
                        continue
                    row = cp.tile([1, HC], BF16, name=f"ar{l}{r}")
                    nc.sync.dma_start(out=row[:], in_=aux_d[l][r:r + 1, :])
                    bc = cp.tile([128, HC], BF16, name=f"ab{l}{r}")
                    nc.gpsimd.partition_broadcast(bc[:], row[:])
                    rows.append(bc)
                aux_b.append(rows)

            # ---- emit helpers ---------------------------------------------
            def phase_a_group(l, g):
                """Tiles 5g..5g+4 (640 padded rows) of phase A for layer l."""
                src_pad = x_in if l == 0 else h_pad
                xT = wp.tile([128, 4, 640], BF16, name="xT", tag="xT")
                for k in range(4):
                    nc.sync.dma_start_transpose(
                        out=xT[:, k, :],
                        in_=src_pad[g * 640:(g + 1) * 640,
                                    k * 128:(k + 1) * 128])
                for t in range(5 * g, 5 * g + 5):
                    off = (t - 5 * g) * 128
                    ps_xl = pp.tile([DBLK, HC], F32, name="ps_xl",
                                    tag="ps_sh", bufs=2)
                    for k in range(4):
                        nc.tensor.matmul(
                            out=ps_xl[:], lhsT=xT[:, k, off:off + DBLK],
                            rhs=wl_t[l][:, k, :],
                            start=(k == 0), stop=(k == 3))
                    xl_o = wp.tile([DBLK, HC], BF16, name="xl_o", tag="xl_o")
                    nc.vector.tensor_add(out=xl_o[:], in0=ps_xl[:],
                                         in1=aux_b[l][0][:DBLK, :])
                    rows = slice(t * DBLK, (t + 1) * DBLK)
                    nc.sync.dma_start(out=xl_sh[l][rows, :], in_=xl_o[:])
                    ps_xr = pp.tile([DBLK, HC], F32, name="ps_xr",
                                    tag="ps_sh", bufs=2)
                    for k in range(4):
                        nc.tensor.matmul(
                            out=ps_xr[:], lhsT=xT[:, k, off:off + DBLK],
                            rhs=wr_t[l][:, k, :],
                            start=(k == 0), stop=(k == 3))
                    xr_o = wp.tile([DBLK, HC], BF16, name="xr_o", tag="xr_o")
                    nc.vector.tensor_add(out=xr_o[:], in0=ps_xr[:],
                                         in1=aux_b[l][1][:DBLK, :])
                    nc.sync.dma_start(out=xr_dr[l][rows, :], in_=xr_o[:])

            def ag_chunk(l, c):
                nc.gpsimd.collective_compute(
                    "AllGather", OP.bypass,
                    replica_groups=[list(range(NCORES))],
                    ins=[xl_sh[l][c * 625:(c + 1) * 625, :]],
                    outs=[xl_full[l][c * NCORES * 625:(c + 1) * NCORES * 625,
                                     :]],
                )

            # halves: half 0 = s in [0, S0), half 1 = s in [S0, S)
            halves = [(0, S0), (S0, S)]

            def edge_A(l, b):
                """Gathers + streams + t-matmuls for block b (both halves).
                Returns per-half state."""
                io0 = b * E16
                xl_gh = []
                for hf, (sa, sb_) in enumerate(halves):
                    nh = (sb_ - sa) * 128
                    xg = gp.tile([128, S0, HC], BF16, name=f"xl_g{hf}",
                                 tag=f"xl_g{hf}")
                    nc.gpsimd.dma_gather(
                        out_ap=xg[:, 0:sb_ - sa, :], in_ap=xl_full[l][:],
                        idxs_ap=si_t[:, io0 + sa * 8:io0 + sa * 8 + nh // 16],
                        num_idxs=nh, num_idxs_reg=nh, elem_size=HC,
                        single_packet=False, queue_num=hf)
                    xl_gh.append(xg)

                oh_b = gp.tile([128, S, DBLK], BF16, name="oh_b", tag="oh_b")
                nc.sync.dma_start(
                    out=oh_b[:],
                    in_=oh_d[:, b * S * DBLK:(b + 1) * S * DBLK])
                ohT_b = gp.tile([DBLK, S, 128], BF16, name="ohT_b",
                                tag="ohT_b")
                nc.sync.dma_start(
                    out=ohT_b[:],
                    in_=ohT_d[:, b * S * 128:(b + 1) * S * 128])
                xr_b = gp.tile([DBLK, HC], BF16, name="xr_b", tag="xr_b")
                nc.sync.dma_start(
                    out=xr_b[:], in_=xr_dr[l][b * DBLK:(b + 1) * DBLK, :])

                ps_lists = [[], []]
                for hf, (sa, sb_) in enumerate(halves):
                    for s2 in range(sa, sb_, 2):
                        w = min(2, sb_ - s2)
                        ps_t = pp.tile([128, 2, HC], F32, name="ps_t",
                                       tag="ps_t", bufs=3)
                        for s in range(s2, s2 + w):
                            nc.tensor.matmul(out=ps_t[:, s - s2, :],
                                             lhsT=ohT_b[:, s, :], rhs=xr_b[:],
                                             start=True, stop=False)
                            nc.tensor.matmul(
                                out=ps_t[:, s - s2, :], lhsT=ident[:],
                                rhs=xl_gh[hf][:, s - sa, :],
                                start=False, stop=True)
                        ps_lists[hf].append((s2, w, ps_t))
                return xl_gh, oh_b, ps_lists

            def edge_evac(l, b, st):
                """ACT LeakyReLU evacuation PSUM -> bf16 SBUF, per half."""
                xl_gh, oh_b, ps_lists = st
                t_lrh = []
                for hf, (sa, sb_) in enumerate(halves):
                    t_lr = gp.tile([128, S0, HC], BF16, name=f"t_lr{hf}",
                                   tag=f"t_lr{hf}")
                    for s2, w, ps_t in ps_lists[hf]:
                        nc.scalar.activation(
                            out=t_lr[:, s2 - sa:s2 - sa + w, :],
                            in_=ps_t[:, 0:w, :], func=ACT.Prelu, alpha=NEG)
                    t_lrh.append(t_lr)
                return t_lrh

            def edge_B1(l, b, st, t_lrh):
                """Per half: logit reduces + exp + value multiply."""
                xl_gh, oh_b, ps_lists = st
                xa = gp.tile([128, S, 2, 258], BF16, name="xa", tag="xa")
                for hf, (sa, sb_) in enumerate(halves):
                    ns = sb_ - sa
                    t_lr = t_lrh[hf]
                    lg_pn = sp.tile([128, 2, S0, H], F32, name=f"lg_pn{hf}",
                                    tag=f"lg_pn{hf}")
                    for h in range(H):
                        np_h = npos[l][h]
                        lo, mid, hi = h * C, h * C + np_h, (h + 1) * C
                        if np_h > 0:
                            nc.vector.tensor_reduce(
                                out=lg_pn[:, 0, 0:ns, h],
                                in_=t_lr[:, 0:ns, lo:mid],
                                axis=AX.X, op=OP.add)
                        else:
                            nc.vector.memset(lg_pn[:, 0, 0:ns, h], 0.0)
                        if np_h < C:
                            nc.vector.tensor_reduce(
                                out=lg_pn[:, 1, 0:ns, h],
                                in_=t_lr[:, 0:ns, mid:hi],
                                axis=AX.X, op=OP.add)
                        else:
                            nc.vector.memset(lg_pn[:, 1, 0:ns, h], 0.0)
                    lg = sp.tile([128, S0, H], F32, name=f"lg{hf}",
                                 tag=f"lg{hf}")
                    nc.vector.tensor_tensor(out=lg[:, 0:ns, :],
                                            in0=lg_pn[:, 0, 0:ns, :],
                                            in1=lg_pn[:, 1, 0:ns, :],
                                            op=OP.subtract)
                    # p = exp(lg): denominator cols + duplicated pairs
                    nc.scalar.activation(
                        out=xa[:, sa:sb_, :, 256:258],
                        in_=lg[:, 0:ns, :].rearrange(
                            "p s (a b) -> p s a b", a=2),
                        func=ACT.Exp)
                    p_dup = sp.tile([128, S0, H, 2], BF16, name=f"p_dup{hf}",
                                    tag=f"p_dup{hf}")
                    nc.scalar.activation(out=p_dup[:, 0:ns, :, 0],
                                         in_=lg[:, 0:ns, :], func=ACT.Exp)
                    nc.scalar.activation(out=p_dup[:, 0:ns, :, 1],
                                         in_=lg[:, 0:ns, :], func=ACT.Exp)
                    # xa = xl * p (packed 2x)
                    for h in range(H):
                        hp, hh = divmod(h, 2)
                        nc.vector.tensor_tensor(
                            out=xa[:, sa:sb_, hp,
                                   hh * 128:(hh + 1) * 128].rearrange(
                                "p s (pr two) -> p s pr two", two=2),
                            in0=xl_gh[hf][:, 0:ns,
                                          h * 128:(h + 1) * 128].rearrange(
                                "p s (pr two) -> p s pr two", two=2),
                            in1=p_dup[:, 0:ns, h, None, :]
                                .to_broadcast([128, ns, 64, 2]),
                            op=OP.mult)
                return xa

            def edge_B2(l, b, st, xa):
                """Value one-hot matmuls + normalize; ELU/store deferred."""
                xl_gh, oh_b, ps_lists = st
                ps_o = [pp.tile([DBLK, HC], F32, name=f"ps_o{hp}",
                                tag="ps_sh", bufs=2) for hp in range(2)]
                for s in range(S):
                    for hp in range(2):
                        nc.tensor.matmul(out=ps_o[hp][:, 0:258],
                                         lhsT=oh_b[:, s, :],
                                         rhs=xa[:, s, hp, :],
                                         start=(s == 0), stop=(s == S - 1))

                rinv = sp.tile([DBLK, 4], F32, name="rinv", tag="rinv")
                for hp in range(2):
                    nc.vector.reciprocal(out=rinv[:, 2 * hp:2 * hp + 2],
                                         in_=ps_o[hp][:, 256:258])
                o_sb = sp.tile([DBLK, HC], BF16 if l == 0 else F32,
                               name="o_sb", tag=f"o_sb{l}")
                for hp in range(2):
                    nc.vector.tensor_tensor(
                        out=o_sb[:, hp * 256:(hp + 1) * 256].rearrange(
                            "p (h c) -> p h c", h=2),
                        in0=ps_o[hp][:, 0:256].rearrange(
                            "p (h c) -> p h c", h=2),
                        in1=rinv[:, 2 * hp:2 * hp + 2][:, :, None]
                            .to_broadcast([DBLK, 2, 128]),
                        op=OP.mult)
                return o_sb

            def edge_B3(l, b, o_sb):
                """ELU (layer 0) and store for block b."""
                if l == 0:
                    nc.vector.tensor_mul(out=o_sb[:], in0=o_sb[:],
                                         in1=aux_b[0][2][:DBLK, :])
                    nc.vector.tensor_add(out=o_sb[:], in0=o_sb[:],
                                         in1=aux_b[0][3][:DBLK, :])
                    r_t = sp.tile([DBLK, HC], BF16, name="r_t", tag="r_t")
                    nc.scalar.activation(out=r_t[:], in_=o_sb[:],
                                         func=ACT.Relu)
                    e_t = sp.tile([DBLK, HC], BF16, name="e_t", tag="e_t")
                    nc.scalar.activation(out=e_t[:], in_=o_sb[:],
                                         func=ACT.Exp)
                    nc.vector.tensor_scalar(
                        out=e_t[:], in0=e_t[:], scalar1=-1.0, scalar2=0.0,
                        op0=OP.add, op1=OP.min)
                    h_t = sp.tile([DBLK, HC], BF16, name="h_t", tag="h_t")
                    nc.vector.tensor_add(out=h_t[:], in0=r_t[:], in1=e_t[:])
                    nc.sync.dma_start(
                        out=h_pad[b * 128:b * 128 + DBLK, :], in_=h_t[:])
                else:
                    nc.sync.dma_start(
                        out=out_d[b * DBLK:(b + 1) * DBLK, :], in_=o_sb[:])

            # ---- schedule (software pipeline per layer) -------------------
            def emit_layer_edges(l, between=None):
                pend_B2 = None   # (b, st, xa)
                pend_B3 = None   # (b, o_sb)
                for b in range(NBLK + 2):
                    st = t_lrh = None
                    if b < NBLK:
                        st = edge_A(l, b)
                        t_lrh = edge_evac(l, b, st)
                    if pend_B3 is not None:
                        edge_B3(l, *pend_B3)
                        if between is not None:
                            between(pend_B3[0])
                        pend_B3 = None
                    if st is not None:
                        xa = edge_B1(l, b, st, t_lrh)
                    if pend_B2 is not None:
                        o_sb = edge_B2(l, pend_B2[0], pend_B2[1],
                                       pend_B2[2])
                        pend_B3 = (pend_B2[0], o_sb)
                        pend_B2 = None
                    if st is not None:
                        pend_B2 = (b, st, xa)

            for g in range(4):
                phase_a_group(0, g)
                ag_chunk(0, g)

            def _between_l0(b):
                if b % 5 == 4:
                    phase_a_group(1, b // 5)
                    ag_chunk(1, b // 5)

            emit_layer_edges(0, between=_between_l0)
            emit_layer_edges(1)

    nc.compile()
    return nc


_CACHE = {}


def _get_nc(e_blk, npos_key):
    key = (e_blk, npos_key)
    if key not in _CACHE:
        _CACHE[key] = _build(e_blk, [list(npos_key[0]), list(npos_key[1])])
    return _CACHE[key]


def kernel(**inputs):
    per_core, meta = _preprocess_graph(np.asarray(inputs["edge_index"]))
    wprep, npos, col_perms, invs = _prep_weights(inputs)
    e_blk = meta["e_blk"]
    perm = meta["perm"]

    nc = _get_nc(e_blk, (tuple(npos[0]), tuple(npos[1])))

    x = np.asarray(inputs["x"], np.float32)
    x_perm = x[perm].astype(bfloat16)
    in_maps = []
    for core in range(NCORES):
        xp = np.zeros((NPAD, IN), bfloat16)
        xc = x_perm[core * NSH:(core + 1) * NSH]
        xp.reshape(NBLK, 128, IN)[:, :DBLK, :] = xc.reshape(NBLK, DBLK, IN)
        m = dict(
            x_pad=xp,
            src_idx=per_core[core]["src_idx"],
            onehot=per_core[core]["onehot"],
            onehotT=per_core[core]["onehotT"],
        )
        for l in range(2):
            m[f"wl{l}"] = wprep[f"wl{l}"]
            m[f"wr{l}"] = wprep[f"wr{l}"]
            m[f"aux{l}"] = wprep[f"aux{l}"]
        in_maps.append(m)

    trace = bool(inputs.pop("_trace", False))
    res = run_bass_kernel_spmd(nc, in_maps, core_ids=list(range(NCORES)),
                               trace=trace)
    out_rows = np.concatenate([res.results[c]["out"] for c in range(NCORES)],
                              axis=0)
    tmp = np.zeros((N, HC), np.float32)
    tmp[perm] = out_rows
    out = np.zeros((N, HC), np.float32)
    bias1 = np.asarray(inputs["bias1"], np.float32)
    out[:, col_perms[1]] = tmp * invs[1][None, :] + bias1[col_perms[1]][None, :]
    if trace:
        kernel._last_result = res
    return out
